# revision 11
# baseline (speedup 1.0000x reference)
"""CondInst dynamic mask head on 8 Trainium2 NeuronCores.

Math (per instance i with gathered params):
    x_i   = [rel_i (2,HW); feats_b (8,HW)]
    h1    = relu(w0_i @ x_i + b0_i)        # (8,HW)
    h2    = relu(w1_i @ h1 + b1_i)         # (8,HW)
    out_i = sigmoid(w2_i @ h2 + b2_i)      # (1,HW)

rel_i is affine in the shared coords map, so it folds into a shared
X = [coords/128; feats; ones] with per-instance effective weights
Ahat_i = [-w0r_i | w0f_i] and bias c0_i riding the ones-row.

Sharding: core c -> batch b=c//2, L-half c%2 (8192 cols), all 100 instances.

Layout: 7 slabs of 16 instances (slab 6 holds 4); per 512-column chunk each
slab runs three full-array 128x128 matmuls (L0 [128,128] zero-padded
stationary, L1 [128,128] block-diagonal, L2 [128,128] sparse whose live
columns sit at the global instance index so all slabs accumulate into one
PSUM bank and output partition p = instance p).  Full-array MMs avoid PE
mode switches and stream at ~216ns/512 cols when not head-blocked.

The roofline is PSUM->SBUF evacuation (ScalarE + VectorE, ~720ns per
[128,512] sustained).  Stages are software-pipelined with 2-iteration lags
so every instruction's dependencies are long done when it reaches its
engine's in-order queue head; evacs are greedily balanced across both
engines.
"""

import os
import sys

import numpy as np

sys.path.insert(0, "/opt/trn_rl_repo")
os.environ.setdefault("MYCRO_LOCAL_CACHE", "1")

B, K, C, H, Wd = 4, 100, 8, 128, 128
HW = H * Wd
LC = HW // 2          # 8192 cols per core
WCH = 512             # chunk (free dim) per matmul / psum bank (512 f32)
NCHUNK = LC // WCH    # 16
NSLAB = 7             # slabs of 16 instances (last has 4)
NCORE = 8
KC0 = 128             # L0 contract (11 live rows zero-padded to 128:
                      # only full-128-contract MMs stream at full rate)

_PROGRAM = None  # cached nc


# ---------------------------------------------------------------- host prep
def _prep_inputs(seg_feat, conv_weight, ind):
    import ml_dtypes
    bf16 = ml_dtypes.bfloat16

    seg_feat = np.asarray(seg_feat, dtype=np.float32)
    conv_weight = np.asarray(conv_weight, dtype=np.float32)
    ind64 = np.asarray(ind).astype(np.int64)

    cw = conv_weight.reshape(B, -1, HW)
    params = np.take_along_axis(cw, ind64[:, None, :], axis=2)  # [B, P, K]
    params = params.transpose(0, 2, 1)  # [B, K, P]

    w0 = params[..., 0:80].reshape(B, K, C, C + 2)
    w1 = params[..., 80:144].reshape(B, K, C, C)
    w2 = params[..., 144:152].reshape(B, K, C)
    b0 = params[..., 152:160]
    b1 = params[..., 160:168]
    b2 = params[..., 168]

    xi = (ind64 % Wd).astype(np.float32)
    yi = (ind64 // Wd).astype(np.float32)
    loc = np.stack([xi, yi], axis=-1)  # [B, K, 2]

    w0r = w0[..., 0:2]   # [B, K, 8, 2]
    w0f = w0[..., 2:10]  # [B, K, 8, 8]
    ahat = np.concatenate([-w0r, w0f], axis=-1)  # [B, K, 8, 10]
    c0 = b0 + np.einsum("bkoc,bkc->bko", w0r, loc) / 128.0  # [B, K, 8]

    lin = np.arange(HW, dtype=np.float32)
    coords_x = (lin % Wd) / 128.0
    coords_y = np.floor(lin / Wd) / 128.0

    in_maps = []
    for core in range(NCORE):
        b = core // 2
        sl = slice((core % 2) * LC, (core % 2) * LC + LC)

        xrep = np.zeros((KC0, LC), dtype=np.float32)
        xrep[0] = coords_x[sl]
        xrep[1] = coords_y[sl]
        xrep[2:10] = seg_feat[b].reshape(C, HW)[:, sl]
        xrep[10] = 1.0

        w0sb = np.zeros((KC0, 128 * NSLAB), dtype=np.float32)
        w1sb = np.zeros((128, 128 * NSLAB), dtype=np.float32)
        w2sb = np.zeros((128, 128 * NSLAB), dtype=np.float32)
        b1sb = np.zeros((128, NSLAB), dtype=np.float32)
        b2sb = np.zeros((128, 1), dtype=np.float32)

        for s in range(NSLAB):
            for q in range(16):          # instance slot within slab
                inst = 16 * s + q
                if inst >= K:
                    continue
                col = 128 * s + 8 * q
                # L0 stationary [32, 128]: rows 0..9 Ahat.T, row 10 c0
                w0sb[0:10, col:col + 8] = ahat[b, inst].T
                w0sb[10, col:col + 8] = c0[b, inst]
                # L1 stationary [128, 128] block-diag: W1^T per instance
                w1sb[8 * q:8 * q + 8, col:col + 8] = w1[b, inst].T
                b1sb[8 * q:8 * q + 8, s] = b1[b, inst]
                # L2 stationary [128, 128]: live col at global inst index
                w2sb[8 * q:8 * q + 8, 128 * s + inst] = w2[b, inst]
        b2sb[:K, 0] = b2[b]

        in_maps.append({
            "xrep": xrep.astype(bf16),
            "w0sb": w0sb.astype(bf16),
            "w1sb": w1sb.astype(bf16),
            "w2sb": w2sb.astype(bf16),
            "b1sb": b1sb, "b2sb": b2sb,
        })
    return in_maps, None


# ---------------------------------------------------------------- program
def build_program():
    global _PROGRAM
    if _PROGRAM is not None:
        return _PROGRAM

    import concourse.tile as tile
    from concourse import bacc, mybir

    nc = bacc.Bacc("TRN2", target_bir_lowering=False, debug=False)
    f32 = mybir.dt.float32
    bf16 = mybir.dt.bfloat16
    Relu = mybir.ActivationFunctionType.Relu
    Sigmoid = mybir.ActivationFunctionType.Sigmoid
    Alu = mybir.AluOpType

    xrep_h = nc.dram_tensor("xrep", [KC0, LC], bf16, kind="ExternalInput")
    w0_h = nc.dram_tensor("w0sb", [KC0, 128 * NSLAB], bf16, kind="ExternalInput")
    w1_h = nc.dram_tensor("w1sb", [128, 128 * NSLAB], bf16, kind="ExternalInput")
    w2_h = nc.dram_tensor("w2sb", [128, 128 * NSLAB], bf16, kind="ExternalInput")
    b1_h = nc.dram_tensor("b1sb", [128, NSLAB], f32, kind="ExternalInput")
    b2_h = nc.dram_tensor("b2sb", [128, 1], f32, kind="ExternalInput")
    out_h = nc.dram_tensor("out_shard", [K, LC], f32, kind="ExternalOutput")

    with tile.TileContext(nc) as tc:
        with (
            tc.tile_pool(name="const", bufs=1) as cpool,
            tc.tile_pool(name="h1p", bufs=4) as h1pool,
            tc.tile_pool(name="h2p", bufs=4) as h2pool,
            tc.tile_pool(name="osp", bufs=2) as ospool,
            tc.tile_pool(name="ps0", bufs=1, space="PSUM") as p0pool,
            tc.tile_pool(name="ps1", bufs=3, space="PSUM") as p1pool,
            tc.tile_pool(name="ps2", bufs=2, space="PSUM") as p2pool,
        ):
            xr = cpool.tile([KC0, LC], bf16, tag="xr")
            w0 = cpool.tile([KC0, 128 * NSLAB], bf16, tag="w0")
            w1 = cpool.tile([128, 128 * NSLAB], bf16, tag="w1")
            w2 = cpool.tile([128, 128 * NSLAB], bf16, tag="w2")
            b1 = cpool.tile([128, NSLAB], f32, tag="b1")
            b2 = cpool.tile([128, 1], f32, tag="b2")

            warm = cpool.tile([1, NSLAB], f32, tag="warm")
            nc.scalar.activation(warm[:], b1[0:1, :], Relu)
            nc.scalar.activation(warm[:], b1[0:1, :], Sigmoid)
            # order matters: w0 + first xr quarter unblock the pipeline;
            # later weights are needed only stages/iterations later
            QW = LC // 4
            nc.gpsimd.dma_start(w0[:], w0_h[:])
            nc.gpsimd.dma_start(xr[:, 0:QW], xrep_h[:, 0:QW])
            nc.gpsimd.dma_start(w1[:], w1_h[:])
            nc.gpsimd.dma_start(w2[:], w2_h[:])
            nc.gpsimd.dma_start(b1[:], b1_h[:])
            nc.gpsimd.dma_start(b2[:], b2_h[:])
            for q in range(1, 4):
                nc.gpsimd.dma_start(xr[:, QW * q:QW * (q + 1)],
                                    xrep_h[:, QW * q:QW * (q + 1)])

            NJOB = NCHUNK * NSLAB  # 112
            p0big = p0pool.tile([128, 3 * WCH], f32, tag="p0big")
            h1big = h1pool.tile([128, 3 * WCH], bf16, tag="h1big")
            p0t, h1t, p1t, h2t = {}, {}, {}, {}
            p2t = {}
            eng_load = {"D": 0.0, "A": 0.0}  # greedy engine balancing

            def evac(dst, src, bias):
                eng = "D" if eng_load["D"] <= eng_load["A"] else "A"
                eng_load[eng] += 1.0
                if eng == "D":
                    if bias is None:
                        nc.vector.tensor_scalar_max(dst, src, 0.0)
                    else:
                        nc.vector.tensor_scalar(dst, src, bias, 0.0,
                                                Alu.add, Alu.max)
                else:
                    if bias is None:
                        nc.scalar.activation(dst, src, Relu)
                    else:
                        nc.scalar.activation(dst, src, Relu, bias=bias)

            # static E1 schedule: pair jobs occupying adjacent slots
            # (slot pattern per 3 jobs: [0,1]=merged, [2]=single); a pair is
            # evacuated one iteration after its later job's L0.
            e1_sched, e1_width = {}, {}
            jj = 0
            while jj < NJOB:
                if jj % 3 < 2 and jj + 1 < NJOB:
                    e1_sched.setdefault(jj + 2, []).append(jj)
                    e1_width[jj] = 2
                    jj += 2
                else:
                    e1_sched.setdefault(jj + 1, []).append(jj)
                    e1_width[jj] = 1
                    jj += 1

            # stage lags: A=0, E1=-1/-2 (pairs), B=-4, E2=-6, C=-8
            for it in range(NJOB + 11):
                # ---- stage C: L2 matmul for job it-8 (oldest deps first)
                j = it - 8
                if 0 <= j < NJOB:
                    k, s = divmod(j, NSLAB)
                    h2 = h2t.pop(j)
                    if s == 0:
                        p2t[k] = p2pool.tile([128, WCH], f32, tag="p2",
                                             name=f"p2_{k}")
                    nc.tensor.matmul(
                        p2t[k][:],
                        w2[:, 128 * s:128 * s + 128],
                        h2[:],
                        start=(s == 0), stop=(s == NSLAB - 1),
                        skip_group_check=True,
                    )

                # ---- sigmoid + store, 2 iterations after a chunk's last L2
                j = it - 10
                if 0 <= j < NJOB and j % NSLAB == NSLAB - 1:
                    k = j // NSLAB
                    fl = slice(WCH * k, WCH * (k + 1))
                    os_t = ospool.tile([128, WCH], f32, tag="os",
                                       name=f"os_{k}")
                    p2 = p2t.pop(k)
                    nc.scalar.activation(os_t[0:K, :], p2[0:K, :],
                                         Sigmoid, bias=b2[0:K, :])
                    eng_load["A"] += 1.0
                    nc.gpsimd.dma_start(out_h[:, fl], os_t[0:K, :])

                # ---- stage B: L1 matmul for job it-4
                j = it - 4
                if 0 <= j < NJOB:
                    k, s = divmod(j, NSLAB)
                    m = j % 3
                    p1 = p1pool.tile([128, WCH], f32, tag="p1",
                                     name=f"p1_{j}")
                    p1t[j] = p1
                    nc.tensor.matmul(
                        p1[:],
                        w1[:, 128 * s:128 * s + 128],
                        h1big[:, WCH * m:WCH * (m + 1)],
                    )

                # ---- stage A: L0 matmul for job `it`
                j = it
                if j < NJOB:
                    k, s = divmod(j, NSLAB)
                    fl = slice(WCH * k, WCH * (k + 1))
                    m = j % 3
                    nc.tensor.matmul(
                        p0big[:, WCH * m:WCH * (m + 1)],
                        w0[:, 128 * s:128 * s + 128],
                        xr[:, fl],
                    )

                # ---- stage E1: evac p0 -> h1 (relu); pairs of jobs whose
                # slots are adjacent (m=0,1 or 1,2) evacuate as one [128,1024]
                for j in e1_sched.get(it, ()):
                    m = j % 3
                    w = e1_width[j]
                    evac(h1big[:, WCH * m:WCH * m + WCH * w],
                         p0big[:, WCH * m:WCH * m + WCH * w], None)

                # ---- stage E2: evac p1 + b1 -> h2 (relu) for job it-5
                j = it - 5
                if 0 <= j < NJOB:
                    k, s = divmod(j, NSLAB)
                    p1 = p1t.pop(j)
                    nr = 128 if s < 6 else 32
                    h2 = h2pool.tile([128, WCH], bf16, tag="h2",
                                     name=f"h2_{j}")
                    h2t[j] = h2
                    evac(h2[0:nr, :], p1[0:nr, :], b1[0:nr, s:s + 1])

    nc.compile()
    _PROGRAM = nc
    return nc


# ---------------------------------------------------------------- entry
def kernel(seg_feat, conv_weight, ind):
    from concourse.bass_utils import run_bass_kernel_spmd

    in_maps, _ = _prep_inputs(seg_feat, conv_weight, ind)
    nc = build_program()
    res = run_bass_kernel_spmd(nc, in_maps, list(range(NCORE)))
    out = np.empty((B, K, HW), dtype=np.float32)
    for core in range(NCORE):
        b = core // 2
        lo = (core % 2) * LC
        out[b, :, lo:lo + LC] = res.results[core]["out_shard"]
    return out.reshape(B, K, H, Wd)


# revision 12
# speedup vs baseline: 1.3739x; 1.3739x over previous
"""CondInst dynamic mask head on 8 Trainium2 NeuronCores.

Math (per instance i with gathered params):
    x_i   = [rel_i (2,HW); feats_b (8,HW)]
    h1    = relu(w0_i @ x_i + b0_i)        # (8,HW)
    h2    = relu(w1_i @ h1 + b1_i)         # (8,HW)
    out_i = sigmoid(w2_i @ h2 + b2_i)      # (1,HW)

rel_i is affine in the shared coords map, so it folds into a shared
X = [coords/128; feats; ones] with per-instance effective weights
Ahat_i = [-w0r_i | w0f_i] and bias c0_i riding the ones-row.

Sharding: core c -> batch b=c//2, L-half c%2 (8192 cols), all 100 instances.

Layout: 7 slabs of 16 instances (slab 6 holds 4); per 512-column chunk each
slab runs three full-array 128x128 matmuls (L0 [128,128] zero-padded
stationary, L1 [128,128] block-diagonal, L2 [128,128] sparse whose live
columns sit at the global instance index so all slabs accumulate into one
PSUM bank and output partition p = instance p).  Full-array MMs avoid PE
mode switches and stream at ~216ns/512 cols when not head-blocked.

The roofline is PSUM->SBUF evacuation (ScalarE + VectorE, ~720ns per
[128,512] sustained).  Stages are software-pipelined with 2-iteration lags
so every instruction's dependencies are long done when it reaches its
engine's in-order queue head; evacs are greedily balanced across both
engines.
"""

import os
import sys

import numpy as np

sys.path.insert(0, "/opt/trn_rl_repo")
os.environ.setdefault("MYCRO_LOCAL_CACHE", "1")

B, K, C, H, Wd = 4, 100, 8, 128, 128
HW = H * Wd
LC = HW // 2          # 8192 cols per core
WCH = 512             # chunk (free dim) per matmul / psum bank (512 f32)
NCHUNK = LC // WCH    # 16
NSLAB = 7             # slabs of 16 instances (last has 4)
NCORE = 8
KC0 = 128             # L0 contract (11 live rows zero-padded to 128:
                      # only full-128-contract MMs stream at full rate)

_PROGRAM = None  # cached nc


# ---------------------------------------------------------------- host prep
def _prep_inputs(seg_feat, conv_weight, ind):
    import ml_dtypes
    bf16 = ml_dtypes.bfloat16

    seg_feat = np.asarray(seg_feat, dtype=np.float32)
    conv_weight = np.asarray(conv_weight, dtype=np.float32)
    ind64 = np.asarray(ind).astype(np.int64)

    cw = conv_weight.reshape(B, -1, HW)
    params = np.take_along_axis(cw, ind64[:, None, :], axis=2)  # [B, P, K]
    params = params.transpose(0, 2, 1)  # [B, K, P]

    w0 = params[..., 0:80].reshape(B, K, C, C + 2)
    w1 = params[..., 80:144].reshape(B, K, C, C)
    w2 = params[..., 144:152].reshape(B, K, C)
    b0 = params[..., 152:160]
    b1 = params[..., 160:168]
    b2 = params[..., 168]

    xi = (ind64 % Wd).astype(np.float32)
    yi = (ind64 // Wd).astype(np.float32)
    loc = np.stack([xi, yi], axis=-1)  # [B, K, 2]

    w0r = w0[..., 0:2]   # [B, K, 8, 2]
    w0f = w0[..., 2:10]  # [B, K, 8, 8]
    ahat = np.concatenate([-w0r, w0f], axis=-1)  # [B, K, 8, 10]
    c0 = b0 + np.einsum("bkoc,bkc->bko", w0r, loc) / 128.0  # [B, K, 8]

    lin = np.arange(HW, dtype=np.float32)
    coords_x = (lin % Wd) / 128.0
    coords_y = np.floor(lin / Wd) / 128.0

    in_maps = []
    for core in range(NCORE):
        b = core // 2
        sl = slice((core % 2) * LC, (core % 2) * LC + LC)

        xrep = np.zeros((KC0, LC), dtype=np.float32)
        xrep[0] = coords_x[sl]
        xrep[1] = coords_y[sl]
        xrep[2:10] = seg_feat[b].reshape(C, HW)[:, sl]
        xrep[10] = 1.0

        w0sb = np.zeros((KC0, 128 * NSLAB), dtype=np.float32)
        w1sb = np.zeros((128, 128 * NSLAB), dtype=np.float32)
        w2sb = np.zeros((128, 128 * NSLAB), dtype=np.float32)
        b1sb = np.zeros((128, NSLAB), dtype=np.float32)
        b2sb = np.zeros((128, 1), dtype=np.float32)

        for s in range(NSLAB):
            for q in range(16):          # instance slot within slab
                inst = 16 * s + q
                if inst >= K:
                    continue
                col = 128 * s + 8 * q
                # L0 stationary [32, 128]: rows 0..9 Ahat.T, row 10 c0
                w0sb[0:10, col:col + 8] = ahat[b, inst].T
                w0sb[10, col:col + 8] = c0[b, inst]
                # L1 stationary [128, 128] block-diag: W1^T per instance
                w1sb[8 * q:8 * q + 8, col:col + 8] = w1[b, inst].T
                b1sb[8 * q:8 * q + 8, s] = b1[b, inst]
                # L2 stationary [128, 128]: live col at global inst index
                w2sb[8 * q:8 * q + 8, 128 * s + inst] = w2[b, inst]
        b2sb[:K, 0] = b2[b]

        in_maps.append({
            "xrep": xrep.astype(bf16),
            "w0sb": w0sb.astype(bf16),
            "w1sb": w1sb.astype(bf16),
            "w2sb": w2sb.astype(bf16),
            "b1sb": b1sb, "b2sb": b2sb,
        })
    return in_maps, None


# ---------------------------------------------------------------- program
def build_program():
    global _PROGRAM
    if _PROGRAM is not None:
        return _PROGRAM

    import concourse.tile as tile
    from concourse import bacc, mybir

    nc = bacc.Bacc("TRN2", target_bir_lowering=False, debug=False)
    f32 = mybir.dt.float32
    bf16 = mybir.dt.bfloat16
    Relu = mybir.ActivationFunctionType.Relu
    Sigmoid = mybir.ActivationFunctionType.Sigmoid
    Alu = mybir.AluOpType

    xrep_h = nc.dram_tensor("xrep", [KC0, LC], bf16, kind="ExternalInput")
    w0_h = nc.dram_tensor("w0sb", [KC0, 128 * NSLAB], bf16, kind="ExternalInput")
    w1_h = nc.dram_tensor("w1sb", [128, 128 * NSLAB], bf16, kind="ExternalInput")
    w2_h = nc.dram_tensor("w2sb", [128, 128 * NSLAB], bf16, kind="ExternalInput")
    b1_h = nc.dram_tensor("b1sb", [128, NSLAB], f32, kind="ExternalInput")
    b2_h = nc.dram_tensor("b2sb", [128, 1], f32, kind="ExternalInput")
    out_h = nc.dram_tensor("out_shard", [K, LC], f32, kind="ExternalOutput")

    with tile.TileContext(nc) as tc:
        with (
            tc.tile_pool(name="const", bufs=1) as cpool,
            tc.tile_pool(name="h1p", bufs=4) as h1pool,
            tc.tile_pool(name="h2p", bufs=4) as h2pool,
            tc.tile_pool(name="osp", bufs=2) as ospool,
            tc.tile_pool(name="ps0", bufs=3, space="PSUM") as p0pool,
            tc.tile_pool(name="ps1", bufs=3, space="PSUM") as p1pool,
            tc.tile_pool(name="ps2", bufs=2, space="PSUM") as p2pool,
        ):
            xr = cpool.tile([KC0, LC], bf16, tag="xr")
            w0 = cpool.tile([KC0, 128 * NSLAB], bf16, tag="w0")
            w1 = cpool.tile([128, 128 * NSLAB], bf16, tag="w1")
            w2 = cpool.tile([128, 128 * NSLAB], bf16, tag="w2")
            b1 = cpool.tile([128, NSLAB], f32, tag="b1")
            b2 = cpool.tile([128, 1], f32, tag="b2")

            warm = cpool.tile([1, NSLAB], f32, tag="warm")
            nc.scalar.activation(warm[:], b1[0:1, :], Relu)
            nc.scalar.activation(warm[:], b1[0:1, :], Sigmoid)
            # order matters: w0 + first xr quarter unblock the pipeline;
            # later weights are needed only stages/iterations later
            QW = LC // 4
            nc.gpsimd.dma_start(w0[:], w0_h[:])
            nc.gpsimd.dma_start(xr[:, 0:QW], xrep_h[:, 0:QW])
            nc.gpsimd.dma_start(w1[:], w1_h[:])
            nc.gpsimd.dma_start(w2[:], w2_h[:])
            nc.gpsimd.dma_start(b1[:], b1_h[:])
            nc.gpsimd.dma_start(b2[:], b2_h[:])
            for q in range(1, 4):
                nc.gpsimd.dma_start(xr[:, QW * q:QW * (q + 1)],
                                    xrep_h[:, QW * q:QW * (q + 1)])

            NJOB = NCHUNK * NSLAB  # 112
            p0t, h1t, p1t, h2t = {}, {}, {}, {}
            p2t = {}
            eng_load = {"D": 0.0, "A": 0.0}  # greedy engine balancing

            def evac(dst, src, bias):
                eng = "D" if eng_load["D"] <= eng_load["A"] else "A"
                eng_load[eng] += 1.0
                if eng == "D":
                    if bias is None:
                        nc.vector.tensor_scalar_max(dst, src, 0.0)
                    else:
                        nc.vector.tensor_scalar(dst, src, bias, 0.0,
                                                Alu.add, Alu.max)
                else:
                    if bias is None:
                        nc.scalar.activation(dst, src, Relu)
                    else:
                        nc.scalar.activation(dst, src, Relu, bias=bias)

            # stage lags: A=0, E1=-2, B=-4, E2=-6, C=-8 (sigmoid rides C)
            for it in range(NJOB + 11):
                # ---- stage C: L2 matmul for job it-8 (oldest deps first)
                j = it - 8
                if 0 <= j < NJOB:
                    k, s = divmod(j, NSLAB)
                    h2 = h2t.pop(j)
                    if s == 0:
                        p2t[k] = p2pool.tile([128, WCH], f32, tag="p2",
                                             name=f"p2_{k}")
                    nc.tensor.matmul(
                        p2t[k][:],
                        w2[:, 128 * s:128 * s + 128],
                        h2[:],
                        start=(s == 0), stop=(s == NSLAB - 1),
                        skip_group_check=True,
                    )

                # ---- sigmoid + store, 2 iterations after a chunk's last L2
                j = it - 10
                if 0 <= j < NJOB and j % NSLAB == NSLAB - 1:
                    k = j // NSLAB
                    fl = slice(WCH * k, WCH * (k + 1))
                    os_t = ospool.tile([128, WCH], f32, tag="os",
                                       name=f"os_{k}")
                    p2 = p2t.pop(k)
                    nc.scalar.activation(os_t[0:K, :], p2[0:K, :],
                                         Sigmoid, bias=b2[0:K, :])
                    eng_load["A"] += 1.0
                    nc.gpsimd.dma_start(out_h[:, fl], os_t[0:K, :])

                # ---- stage B: L1 matmul for job it-4
                j = it - 4
                if 0 <= j < NJOB:
                    k, s = divmod(j, NSLAB)
                    h1 = h1t.pop(j)
                    p1 = p1pool.tile([128, WCH], f32, tag="p1",
                                     name=f"p1_{j}")
                    p1t[j] = p1
                    nc.tensor.matmul(
                        p1[:],
                        w1[:, 128 * s:128 * s + 128],
                        h1[:],
                    )

                # ---- stage A: L0 matmul for job `it`
                j = it
                if j < NJOB:
                    k, s = divmod(j, NSLAB)
                    fl = slice(WCH * k, WCH * (k + 1))
                    p0 = p0pool.tile([128, WCH], f32, tag="p0",
                                     name=f"p0_{j}")
                    p0t[j] = p0
                    nc.tensor.matmul(
                        p0[:],
                        w0[:, 128 * s:128 * s + 128],
                        xr[:, fl],
                    )

                # ---- stage E1: evac p0 -> h1 (relu) for job it-1
                j = it - 1
                if 0 <= j < NJOB:
                    k, s = divmod(j, NSLAB)
                    p0 = p0t.pop(j)
                    nr = 128 if s < 6 else 32
                    h1 = h1pool.tile([128, WCH], bf16, tag="h1",
                                     name=f"h1_{j}")
                    h1t[j] = h1
                    evac(h1[0:nr, :], p0[0:nr, :], None)

                # ---- stage E2: evac p1 + b1 -> h2 (relu) for job it-5
                j = it - 5
                if 0 <= j < NJOB:
                    k, s = divmod(j, NSLAB)
                    p1 = p1t.pop(j)
                    nr = 128 if s < 6 else 32
                    h2 = h2pool.tile([128, WCH], bf16, tag="h2",
                                     name=f"h2_{j}")
                    h2t[j] = h2
                    evac(h2[0:nr, :], p1[0:nr, :], b1[0:nr, s:s + 1])

    nc.compile()
    _PROGRAM = nc
    return nc


# ---------------------------------------------------------------- entry
def kernel(seg_feat, conv_weight, ind):
    from concourse.bass_utils import run_bass_kernel_spmd

    in_maps, _ = _prep_inputs(seg_feat, conv_weight, ind)
    nc = build_program()
    res = run_bass_kernel_spmd(nc, in_maps, list(range(NCORE)))
    out = np.empty((B, K, HW), dtype=np.float32)
    for core in range(NCORE):
        b = core // 2
        lo = (core % 2) * LC
        out[b, :, lo:lo + LC] = res.results[core]["out_shard"]
    return out.reshape(B, K, H, Wd)


# revision 14
# speedup vs baseline: 1.4114x; 1.0273x over previous
"""CondInst dynamic mask head on 8 Trainium2 NeuronCores.

Math (per instance i with gathered params):
    x_i   = [rel_i (2,HW); feats_b (8,HW)]
    h1    = relu(w0_i @ x_i + b0_i)        # (8,HW)
    h2    = relu(w1_i @ h1 + b1_i)         # (8,HW)
    out_i = sigmoid(w2_i @ h2 + b2_i)      # (1,HW)

rel_i is affine in the shared coords map, so it folds into a shared
X = [coords/128; feats; ones] with per-instance effective weights
Ahat_i = [-w0r_i | w0f_i] and bias c0_i riding the ones-row.

Sharding: core c -> batch b=c//2, L-half c%2 (8192 cols), all 100 instances.

Layout: 7 slabs of 16 instances (slab 6 holds 4); per 512-column chunk each
slab runs three full-array 128x128 matmuls (L0 [128,128] zero-padded
stationary, L1 [128,128] block-diagonal, L2 [128,128] sparse whose live
columns sit at the global instance index so all slabs accumulate into one
PSUM bank and output partition p = instance p).  Full-array MMs avoid PE
mode switches and stream at ~216ns/512 cols when not head-blocked.

The roofline is PSUM->SBUF evacuation (ScalarE + VectorE, ~720ns per
[128,512] sustained).  Stages are software-pipelined with 2-iteration lags
so every instruction's dependencies are long done when it reaches its
engine's in-order queue head; evacs are greedily balanced across both
engines.
"""

import os
import sys

import numpy as np

sys.path.insert(0, "/opt/trn_rl_repo")
os.environ.setdefault("MYCRO_LOCAL_CACHE", "1")

B, K, C, H, Wd = 4, 100, 8, 128, 128
HW = H * Wd
LC = HW // 2          # 8192 cols per core
WCH = 512             # chunk (free dim) per matmul / psum bank (512 f32)
NCHUNK = LC // WCH    # 16
NSLAB = 7             # slabs of 16 instances (last has 4)
NCORE = 8
KC0 = 128             # L0 contract (11 live rows zero-padded to 128:
                      # only full-128-contract MMs stream at full rate)

_PROGRAM = None  # cached nc


# ---------------------------------------------------------------- host prep
def _prep_inputs(seg_feat, conv_weight, ind):
    import ml_dtypes
    bf16 = ml_dtypes.bfloat16

    seg_feat = np.asarray(seg_feat, dtype=np.float32)
    conv_weight = np.asarray(conv_weight, dtype=np.float32)
    ind64 = np.asarray(ind).astype(np.int64)

    cw = conv_weight.reshape(B, -1, HW)
    params = np.take_along_axis(cw, ind64[:, None, :], axis=2)  # [B, P, K]
    params = params.transpose(0, 2, 1)  # [B, K, P]

    w0 = params[..., 0:80].reshape(B, K, C, C + 2)
    w1 = params[..., 80:144].reshape(B, K, C, C)
    w2 = params[..., 144:152].reshape(B, K, C)
    b0 = params[..., 152:160]
    b1 = params[..., 160:168]
    b2 = params[..., 168]

    xi = (ind64 % Wd).astype(np.float32)
    yi = (ind64 // Wd).astype(np.float32)
    loc = np.stack([xi, yi], axis=-1)  # [B, K, 2]

    w0r = w0[..., 0:2]   # [B, K, 8, 2]
    w0f = w0[..., 2:10]  # [B, K, 8, 8]
    ahat = np.concatenate([-w0r, w0f], axis=-1)  # [B, K, 8, 10]
    c0 = b0 + np.einsum("bkoc,bkc->bko", w0r, loc) / 128.0  # [B, K, 8]

    lin = np.arange(HW, dtype=np.float32)
    coords_x = (lin % Wd) / 128.0
    coords_y = np.floor(lin / Wd) / 128.0

    in_maps = []
    for core in range(NCORE):
        b = core // 2
        sl = slice((core % 2) * LC, (core % 2) * LC + LC)

        xrep = np.zeros((KC0, LC), dtype=np.float32)
        xrep[0] = coords_x[sl]
        xrep[1] = coords_y[sl]
        xrep[2:10] = seg_feat[b].reshape(C, HW)[:, sl]
        xrep[10] = 1.0

        w0sb = np.zeros((KC0, 128 * NSLAB), dtype=np.float32)
        w1sb = np.zeros((128, 128 * NSLAB), dtype=np.float32)
        w2sb = np.zeros((128, 128 * NSLAB), dtype=np.float32)
        b1sb = np.zeros((128, NSLAB), dtype=np.float32)
        b2sb = np.zeros((128, 1), dtype=np.float32)

        for s in range(NSLAB):
            for q in range(16):          # instance slot within slab
                inst = 16 * s + q
                if inst >= K:
                    continue
                col = 128 * s + 8 * q
                # L0 stationary [128, 128]: rows 0..9 Ahat.T, row 10 c0
                w0sb[0:10, col:col + 8] = ahat[b, inst].T
                w0sb[10, col:col + 8] = c0[b, inst]
                # L1 stationary [128, 128] block-diag: W1^T per instance
                w1sb[8 * q:8 * q + 8, col:col + 8] = w1[b, inst].T
                b1sb[8 * q:8 * q + 8, s] = b1[b, inst]
                # L2 stationary [128, 128]: live col at global inst index
                w2sb[8 * q:8 * q + 8, 128 * s + inst] = w2[b, inst]
        b2sb[:K, 0] = b2[b]

        in_maps.append({
            "xrep": xrep.astype(bf16),
            "w0sb": w0sb.astype(bf16),
            "w1sb": w1sb.astype(bf16),
            "w2sb": w2sb.astype(bf16),
            "b1sb": b1sb, "b2sb": b2sb,
        })
    return in_maps, None


# ---------------------------------------------------------------- program
def build_program():
    global _PROGRAM
    if _PROGRAM is not None:
        return _PROGRAM

    import concourse.tile as tile
    from concourse import bacc, mybir

    nc = bacc.Bacc("TRN2", target_bir_lowering=False, debug=False)
    f32 = mybir.dt.float32
    bf16 = mybir.dt.bfloat16
    Relu = mybir.ActivationFunctionType.Relu
    Sigmoid = mybir.ActivationFunctionType.Sigmoid
    Alu = mybir.AluOpType

    xrep_h = nc.dram_tensor("xrep", [KC0, LC], bf16, kind="ExternalInput")
    w0_h = nc.dram_tensor("w0sb", [KC0, 128 * NSLAB], bf16, kind="ExternalInput")
    w1_h = nc.dram_tensor("w1sb", [128, 128 * NSLAB], bf16, kind="ExternalInput")
    w2_h = nc.dram_tensor("w2sb", [128, 128 * NSLAB], bf16, kind="ExternalInput")
    b1_h = nc.dram_tensor("b1sb", [128, NSLAB], f32, kind="ExternalInput")
    b2_h = nc.dram_tensor("b2sb", [128, 1], f32, kind="ExternalInput")
    out_h = nc.dram_tensor("out_shard", [K, LC], f32, kind="ExternalOutput")

    with tile.TileContext(nc) as tc:
        with (
            tc.tile_pool(name="const", bufs=1) as cpool,
            tc.tile_pool(name="h1p", bufs=5) as h1pool,
            tc.tile_pool(name="h2p", bufs=5) as h2pool,
            tc.tile_pool(name="osp", bufs=2) as ospool,
            tc.tile_pool(name="ps0", bufs=3, space="PSUM") as p0pool,
            tc.tile_pool(name="ps1", bufs=3, space="PSUM") as p1pool,
            tc.tile_pool(name="ps2", bufs=2, space="PSUM") as p2pool,
        ):
            xr = cpool.tile([KC0, LC], bf16, tag="xr")
            w0 = cpool.tile([KC0, 128 * NSLAB], bf16, tag="w0")
            w1 = cpool.tile([128, 128 * NSLAB], bf16, tag="w1")
            w2 = cpool.tile([128, 128 * NSLAB], bf16, tag="w2")
            b1 = cpool.tile([128, NSLAB], f32, tag="b1")
            b2 = cpool.tile([128, 1], f32, tag="b2")

            warm = cpool.tile([1, NSLAB], f32, tag="warm")
            nc.scalar.activation(warm[:], b1[0:1, :], Relu)
            nc.scalar.activation(warm[:], b1[0:1, :], Sigmoid)
            # order matters: w0 + first xr quarter unblock the pipeline;
            # later weights are needed only stages/iterations later
            QW = LC // 4
            nc.gpsimd.dma_start(xr[:, 0:WCH], xrep_h[:, 0:WCH])
            nc.gpsimd.dma_start(w0[:], w0_h[:])
            nc.gpsimd.dma_start(xr[:, WCH:QW], xrep_h[:, WCH:QW])
            nc.gpsimd.dma_start(w1[:], w1_h[:])
            nc.gpsimd.dma_start(w2[:], w2_h[:])
            nc.gpsimd.dma_start(b1[:], b1_h[:])
            nc.gpsimd.dma_start(b2[:], b2_h[:])
            for q in range(1, 4):
                nc.gpsimd.dma_start(xr[:, QW * q:QW * (q + 1)],
                                    xrep_h[:, QW * q:QW * (q + 1)])

            NJOB = NCHUNK * NSLAB  # 112
            p0t, h1t, p1t, h2t = {}, {}, {}, {}
            p2t = {}
            eng_load = {"D": 0.0, "A": 0.0}  # greedy engine balancing

            def evac(dst, src, bias):
                eng = "D" if eng_load["D"] + 0.706 <= eng_load["A"] + 0.688 \
                    else "A"
                eng_load[eng] += 0.706 if eng == "D" else 0.688
                if eng == "D":
                    if bias is None:
                        nc.vector.tensor_scalar_max(dst, src, 0.0)
                    else:
                        nc.vector.tensor_scalar(dst, src, bias, 0.0,
                                                Alu.add, Alu.max)
                else:
                    if bias is None:
                        nc.scalar.activation(dst, src, Relu)
                    else:
                        nc.scalar.activation(dst, src, Relu, bias=bias)

            # stage lags: A=0, E1=-2, B=-4, E2=-6, C=-8 (sigmoid rides C)
            for it in range(NJOB + 11):
                # ---- stage C: L2 matmul for job it-8 (oldest deps first)
                j = it - 8
                if 0 <= j < NJOB:
                    k, s = divmod(j, NSLAB)
                    h2 = h2t.pop(j)
                    if s == 0:
                        p2t[k] = p2pool.tile([128, WCH], f32, tag="p2",
                                             name=f"p2_{k}")
                    nc.tensor.matmul(
                        p2t[k][:],
                        w2[:, 128 * s:128 * s + 128],
                        h2[:],
                        start=(s == 0), stop=(s == NSLAB - 1),
                        skip_group_check=True,
                    )

                # ---- sigmoid + store, 2 iterations after a chunk's last L2
                j = it - 10
                if 0 <= j < NJOB and j % NSLAB == NSLAB - 1:
                    k = j // NSLAB
                    fl = slice(WCH * k, WCH * (k + 1))
                    os_t = ospool.tile([128, WCH], f32, tag="os",
                                       name=f"os_{k}")
                    p2 = p2t.pop(k)
                    nc.scalar.activation(os_t[0:K, :], p2[0:K, :],
                                         Sigmoid, bias=b2[0:K, :])
                    eng_load["A"] += 0.688
                    nc.gpsimd.dma_start(out_h[:, fl], os_t[0:K, :])

                # ---- stage B: L1 matmul for job it-4
                j = it - 4
                if 0 <= j < NJOB:
                    k, s = divmod(j, NSLAB)
                    h1 = h1t.pop(j)
                    p1 = p1pool.tile([128, WCH], f32, tag="p1",
                                     name=f"p1_{j}")
                    p1t[j] = p1
                    nc.tensor.matmul(
                        p1[:],
                        w1[:, 128 * s:128 * s + 128],
                        h1[:],
                    )

                # ---- stage A: L0 matmul for job `it`
                j = it
                if j < NJOB:
                    k, s = divmod(j, NSLAB)
                    fl = slice(WCH * k, WCH * (k + 1))
                    p0 = p0pool.tile([128, WCH], f32, tag="p0",
                                     name=f"p0_{j}")
                    p0t[j] = p0
                    nc.tensor.matmul(
                        p0[:],
                        w0[:, 128 * s:128 * s + 128],
                        xr[:, fl],
                    )

                # ---- stage E1: evac p0 -> h1 (relu) for job it-1
                j = it - 1
                if 0 <= j < NJOB:
                    k, s = divmod(j, NSLAB)
                    p0 = p0t.pop(j)
                    nr = 128 if s < 6 else 32
                    h1 = h1pool.tile([128, WCH], bf16, tag="h1",
                                     name=f"h1_{j}")
                    h1t[j] = h1
                    evac(h1[0:nr, :], p0[0:nr, :], None)

                # ---- stage E2: evac p1 + b1 -> h2 (relu) for job it-5
                j = it - 5
                if 0 <= j < NJOB:
                    k, s = divmod(j, NSLAB)
                    p1 = p1t.pop(j)
                    nr = 128 if s < 6 else 32
                    h2 = h2pool.tile([128, WCH], bf16, tag="h2",
                                     name=f"h2_{j}")
                    h2t[j] = h2
                    evac(h2[0:nr, :], p1[0:nr, :], b1[0:nr, s:s + 1])

    nc.compile()
    _PROGRAM = nc
    return nc


# ---------------------------------------------------------------- entry
def kernel(seg_feat, conv_weight, ind):
    from concourse.bass_utils import run_bass_kernel_spmd

    in_maps, _ = _prep_inputs(seg_feat, conv_weight, ind)
    nc = build_program()
    res = run_bass_kernel_spmd(nc, in_maps, list(range(NCORE)))
    out = np.empty((B, K, HW), dtype=np.float32)
    for core in range(NCORE):
        b = core // 2
        lo = (core % 2) * LC
        out[b, :, lo:lo + LC] = res.results[core]["out_shard"]
    return out.reshape(B, K, H, Wd)
